# revision 8
# baseline (speedup 1.0000x reference)
"""Trainium2 Bass kernel for nn_Decoder (GNN edge decoder / link predictor).

Math (per edge e with endpoints src[e], tar[e]):
    h   = relu(x[src] @ W1[:D] + x[tar] @ W1[D:] + b1)        # [E, D]
    out = h @ W2 + b2                                          # [E, 1]

Strategy (8 NeuronCores, SPMD):
  - Shard the 524288 edges across 8 cores (65536 edges each); replicate x
    and the MLP weights. No collectives.
  - Per core, loop over 128 "superblocks" of 512 edges:
      * indirect-DMA gather x[src] and x[tar] rows (128 rows/partition-major
        tile, 4 blocks of 128 edges)
      * PE-transpose each 128x128 block -> xT layout [feature, edge]
      * hT = W1a.T @ xsT + W1b.T @ xtT   (weights stationary, PSUM accum)
      * relu(hT + b1) fused on ACT (per-partition bias)
      * scores = W2.T @ relu_hT  (PE, M=1) ; + b2 on ACT
      * DMA scores back to DRAM
Host reorders edge indices so each core's output is contiguous and in
original edge order.
"""

import sys
from contextlib import ExitStack

import numpy as np

if "/opt/trn_rl_repo" not in sys.path:
    sys.path.insert(0, "/opt/trn_rl_repo")

N_NODES = 100000
D = 128
E_TOTAL = 524288
N_CORES = 8
E_PER_CORE = E_TOTAL // N_CORES  # 65536
SB = 512  # edges per superblock
P = 128


def build_nc(n_sb=E_PER_CORE // SB, repeat=1, x_external=True):
    """Build the single-core Bass program (SPMD: same program, per-core data).

    repeat>1 re-runs the whole edge workload (same outputs) for steady-state
    HW timing via the delta method; results are identical.
    x_external=False makes x an internal (uninitialized) DRAM tensor so the
    timing harness doesn't ship 51MB/core per call; timing is data-independent.
    """
    import concourse.bacc as bacc
    import concourse.bass as bass
    import concourse.mybir as mybir
    import concourse.tile as tile
    from concourse.masks import make_identity

    f32 = mybir.dt.float32
    i32 = mybir.dt.int32
    FT = mybir.ActivationFunctionType

    nc = bacc.Bacc("TRN2", target_bir_lowering=False, debug=False)
    if x_external:
        x_d = nc.dram_tensor("x", [N_NODES, D], f32, kind="ExternalInput")
    else:
        x_d = nc.dram_tensor("x", [N_NODES, D], f32)
    # idx layout: [p, sb*4 + c] = node index for edge  sb*512 + c*128 + p
    src_d = nc.dram_tensor("src", [P, n_sb * 4], i32, kind="ExternalInput")
    tar_d = nc.dram_tensor("tar", [P, n_sb * 4], i32, kind="ExternalInput")
    w1a_d = nc.dram_tensor("w1a", [D, D], f32, kind="ExternalInput")
    w1b_d = nc.dram_tensor("w1b", [D, D], f32, kind="ExternalInput")
    b1_d = nc.dram_tensor("b1", [D, 1], f32, kind="ExternalInput")
    w2_d = nc.dram_tensor("w2", [D, 1], f32, kind="ExternalInput")
    b2_d = nc.dram_tensor("b2", [1, 1], f32, kind="ExternalInput")
    out_d = nc.dram_tensor("out", [n_sb, SB], f32, kind="ExternalOutput")

    with tile.TileContext(nc) as tc, ExitStack() as ctx:
        const = ctx.enter_context(tc.tile_pool(name="const", bufs=1))
        gpool = ctx.enter_context(tc.tile_pool(name="gath", bufs=3))
        tpool = ctx.enter_context(tc.tile_pool(name="xT", bufs=3))
        hpool = ctx.enter_context(tc.tile_pool(name="h", bufs=3))
        spool = ctx.enter_context(tc.tile_pool(name="s", bufs=4))
        psT = ctx.enter_context(tc.tile_pool(name="psT", bufs=2, space="PSUM"))
        psH = ctx.enter_context(tc.tile_pool(name="psH", bufs=2, space="PSUM"))
        psS = ctx.enter_context(tc.tile_pool(name="psS", bufs=2, space="PSUM"))

        ident = const.tile([P, P], f32)
        make_identity(nc, ident[:])
        w1a_t = const.tile([D, D], f32)
        nc.sync.dma_start(w1a_t[:], w1a_d[:, :])
        w1b_t = const.tile([D, D], f32)
        nc.sync.dma_start(w1b_t[:], w1b_d[:, :])
        b1_t = const.tile([D, 1], f32)
        nc.sync.dma_start(b1_t[:], b1_d[:, :])
        w2_t = const.tile([D, 1], f32)
        nc.sync.dma_start(w2_t[:], w2_d[:, :])
        b2_t = const.tile([1, 1], f32)
        nc.sync.dma_start(b2_t[:], b2_d[:, :])
        src_t = const.tile([P, n_sb * 4], i32)
        nc.sync.dma_start(src_t[:], src_d[:, :])
        tar_t = const.tile([P, n_sb * 4], i32)
        nc.sync.dma_start(tar_t[:], tar_d[:, :])

        for sb_rep in range(n_sb * repeat):
            sb = sb_rep % n_sb
            xs = gpool.tile([P, 4, D], f32, tag="xs")
            xt = gpool.tile([P, 4, D], f32, tag="xt")
            for c in range(4):
                nc.gpsimd.indirect_dma_start(
                    out=xs[:, c, :],
                    out_offset=None,
                    in_=x_d[:, :],
                    in_offset=bass.IndirectOffsetOnAxis(
                        ap=src_t[:, sb * 4 + c : sb * 4 + c + 1], axis=0
                    ),
                )
                nc.gpsimd.indirect_dma_start(
                    out=xt[:, c, :],
                    out_offset=None,
                    in_=x_d[:, :],
                    in_offset=bass.IndirectOffsetOnAxis(
                        ap=tar_t[:, sb * 4 + c : sb * 4 + c + 1], axis=0
                    ),
                )
            xsT_ps = psT.tile([P, SB], f32, tag="xsT")
            xtT_ps = psT.tile([P, SB], f32, tag="xtT")
            for c in range(4):
                nc.tensor.transpose(
                    out=xsT_ps[:, c * P : (c + 1) * P], in_=xs[:, c, :], identity=ident[:]
                )
                nc.tensor.transpose(
                    out=xtT_ps[:, c * P : (c + 1) * P], in_=xt[:, c, :], identity=ident[:]
                )
            xsT = tpool.tile([P, SB], f32, tag="xsTs")
            xtT = tpool.tile([P, SB], f32, tag="xtTs")
            nc.vector.tensor_copy(xsT[:], xsT_ps[:])
            nc.vector.tensor_copy(xtT[:], xtT_ps[:])

            h_ps = psH.tile([P, SB], f32, tag="h")
            nc.tensor.matmul(h_ps[:], lhsT=w1a_t[:], rhs=xsT[:], start=True, stop=False)
            nc.tensor.matmul(h_ps[:], lhsT=w1b_t[:], rhs=xtT[:], start=False, stop=True)

            hrelu = hpool.tile([P, SB], f32, tag="hrelu")
            nc.scalar.activation(
                out=hrelu[:], in_=h_ps[:], func=FT.Relu, bias=b1_t[:, 0:1]
            )

            s_ps = psS.tile([1, SB], f32, tag="s")
            nc.tensor.matmul(s_ps[:], lhsT=w2_t[:], rhs=hrelu[:], start=True, stop=True)

            s_sb = spool.tile([1, SB], f32, tag="sout")
            nc.scalar.activation(
                out=s_sb[:], in_=s_ps[:], func=FT.Identity, bias=b2_t[0:1, 0:1]
            )
            nc.sync.dma_start(out_d[sb : sb + 1, :], s_sb[0:1, :])

    nc.compile()
    return nc


def _permute_idx(idx, n_sb):
    """[n_sb*512] -> [128, n_sb*4] with [p, sb*4+c] = idx[sb*512 + c*128 + p]."""
    return np.ascontiguousarray(
        idx.reshape(n_sb, 4, P).transpose(2, 0, 1).reshape(P, n_sb * 4)
    )


_CACHE = {}


def kernel(**inputs):
    x = np.ascontiguousarray(np.asarray(inputs["x"], dtype=np.float32))
    pos = np.asarray(inputs["pos_edge_index"])
    neg = np.asarray(inputs["neg_edge_index"])
    W1 = np.asarray(inputs["W1"], dtype=np.float32)
    b1 = np.asarray(inputs["b1"], dtype=np.float32)
    W2 = np.asarray(inputs["W2"], dtype=np.float32)
    b2 = np.asarray(inputs["b2"], dtype=np.float32)

    edge = np.concatenate([pos, neg], axis=1).astype(np.int32)  # [2, E_TOTAL]
    src, tar = edge[0], edge[1]
    n_sb = E_PER_CORE // SB

    if "nc" not in _CACHE:
        _CACHE["nc"] = build_nc(n_sb)
    nc = _CACHE["nc"]

    w1a = np.ascontiguousarray(W1[:D, :])
    w1b = np.ascontiguousarray(W1[D:, :])
    b1c = np.ascontiguousarray(b1.reshape(D, 1))
    w2c = np.ascontiguousarray(W2.reshape(D, 1))
    b2c = np.ascontiguousarray(b2.reshape(1, 1))

    in_maps = []
    for c in range(N_CORES):
        lo, hi = c * E_PER_CORE, (c + 1) * E_PER_CORE
        in_maps.append(
            {
                "x": x,
                "src": _permute_idx(src[lo:hi], n_sb),
                "tar": _permute_idx(tar[lo:hi], n_sb),
                "w1a": w1a,
                "w1b": w1b,
                "b1": b1c,
                "w2": w2c,
                "b2": b2c,
            }
        )

    from concourse.bass_utils import run_bass_kernel_spmd

    res = run_bass_kernel_spmd(nc, in_maps, list(range(N_CORES))).results
    out = np.concatenate([res[c]["out"].reshape(-1) for c in range(N_CORES)])
    return out.reshape(E_TOTAL, 1).astype(np.float32)


if __name__ == "__main__":
    # smoke test with random data
    rng = np.random.default_rng(0)
    ins = {
        "x": rng.standard_normal((N_NODES, D), dtype=np.float32),
        "pos_edge_index": rng.integers(0, N_NODES, (2, E_TOTAL // 2)),
        "neg_edge_index": rng.integers(0, N_NODES, (2, E_TOTAL // 2)),
        "W1": rng.standard_normal((2 * D, D), dtype=np.float32) * 0.06,
        "b1": rng.standard_normal(D, dtype=np.float32) * 0.06,
        "W2": rng.standard_normal((D, 1), dtype=np.float32) * 0.09,
        "b2": rng.standard_normal(1, dtype=np.float32) * 0.09,
    }
    out = kernel(**ins)
    h = np.maximum(
        ins["x"][np.concatenate([ins["pos_edge_index"][0], ins["neg_edge_index"][0]])]
        @ ins["W1"][:D]
        + ins["x"][np.concatenate([ins["pos_edge_index"][1], ins["neg_edge_index"][1]])]
        @ ins["W1"][D:]
        + ins["b1"],
        0.0,
    )
    exp = h @ ins["W2"] + ins["b2"]
    err = np.abs(out - exp).max() / max(np.abs(exp).max(), 1e-9)
    print("max rel err:", err)


# revision 12
# speedup vs baseline: 7.3351x; 7.3351x over previous
"""Trainium2 Bass kernel for nn_Decoder (GNN edge decoder / link predictor).

Math (per edge e with endpoints src[e], tar[e]):
    h   = relu(x[src] @ W1[:D] + x[tar] @ W1[D:] + b1)        # [E, D]
    out = h @ W2 + b2                                          # [E, 1]

Strategy (8 NeuronCores, SPMD):
  - Shard the 524288 edges across 8 cores (65536 edges each); replicate x
    and the MLP weights. No collectives.
  - Per core, loop over 128 "superblocks" of 512 edges:
      * indirect-DMA gather x[src] and x[tar] rows (128 rows/partition-major
        tile, 4 blocks of 128 edges)
      * PE-transpose each 128x128 block -> xT layout [feature, edge]
      * hT = W1a.T @ xsT + W1b.T @ xtT   (weights stationary, PSUM accum)
      * relu(hT + b1) fused on ACT (per-partition bias)
      * scores = W2.T @ relu_hT  (PE, M=1) ; + b2 on ACT
      * DMA scores back to DRAM
Host reorders edge indices so each core's output is contiguous and in
original edge order.
"""

import sys
from contextlib import ExitStack

import numpy as np

if "/opt/trn_rl_repo" not in sys.path:
    sys.path.insert(0, "/opt/trn_rl_repo")

N_NODES = 100000
D = 128
E_TOTAL = 524288
N_CORES = 8
E_PER_CORE = E_TOTAL // N_CORES  # 65536
SB = 512  # edges per superblock
P = 128


def build_nc(n_sb=E_PER_CORE // SB, repeat=1, x_external=True):
    """Build the single-core Bass program (SPMD: same program, per-core data).

    repeat>1 re-runs the whole edge workload (same outputs) for steady-state
    HW timing via the delta method; results are identical.
    x_external=False makes x an internal (uninitialized) DRAM tensor so the
    timing harness doesn't ship 51MB/core per call; timing is data-independent.
    """
    return _build_nc_impl(n_sb, repeat, x_external, hw_loop=0)


def _build_nc_impl(n_sb, repeat, x_external, hw_loop):
    import concourse.bacc as bacc
    import concourse.bass as bass
    import concourse.mybir as mybir
    import concourse.tile as tile
    from concourse.masks import make_identity
    from contextlib import ExitStack, nullcontext

    f32 = mybir.dt.float32
    i32 = mybir.dt.int32
    FT = mybir.ActivationFunctionType

    nc = bacc.Bacc("TRN2", target_bir_lowering=False, debug=False)
    if x_external:
        x_d = nc.dram_tensor("x", [N_NODES, D], f32, kind="ExternalInput")
    else:
        x_d = nc.dram_tensor("x", [N_NODES, D], f32)
    # idx layout: [p, sb*4 + c] = node index for edge  sb*512 + c*128 + p
    src_d = nc.dram_tensor("src", [P, n_sb * 4], i32, kind="ExternalInput")
    tar_d = nc.dram_tensor("tar", [P, n_sb * 4], i32, kind="ExternalInput")
    w1a_d = nc.dram_tensor("w1a", [D, D], f32, kind="ExternalInput")
    w1b_d = nc.dram_tensor("w1b", [D, D], f32, kind="ExternalInput")
    b1_d = nc.dram_tensor("b1", [D, 1], f32, kind="ExternalInput")
    w2_d = nc.dram_tensor("w2", [D, 1], f32, kind="ExternalInput")
    b2_d = nc.dram_tensor("b2", [1, 1], f32, kind="ExternalInput")
    out_d = nc.dram_tensor("out", [n_sb, SB], f32, kind="ExternalOutput")

    with tile.TileContext(nc) as tc, ExitStack() as ctx:
        const = ctx.enter_context(tc.tile_pool(name="const", bufs=1))
        gpool = ctx.enter_context(tc.tile_pool(name="gath", bufs=3))
        tpool = ctx.enter_context(tc.tile_pool(name="xT", bufs=3))
        hpool = ctx.enter_context(tc.tile_pool(name="h", bufs=3))
        spool = ctx.enter_context(tc.tile_pool(name="s", bufs=4))
        psT = ctx.enter_context(tc.tile_pool(name="psT", bufs=2, space="PSUM"))
        psH = ctx.enter_context(tc.tile_pool(name="psH", bufs=2, space="PSUM"))
        psS = ctx.enter_context(tc.tile_pool(name="psS", bufs=2, space="PSUM"))

        ident = const.tile([P, P], f32)
        make_identity(nc, ident[:])
        w1a_t = const.tile([D, D], f32)
        nc.sync.dma_start(w1a_t[:], w1a_d[:, :])
        w1b_t = const.tile([D, D], f32)
        nc.sync.dma_start(w1b_t[:], w1b_d[:, :])
        b1_t = const.tile([D, 1], f32)
        nc.sync.dma_start(b1_t[:], b1_d[:, :])
        w2_t = const.tile([D, 1], f32)
        nc.sync.dma_start(w2_t[:], w2_d[:, :])
        b2_t = const.tile([1, 1], f32)
        nc.sync.dma_start(b2_t[:], b2_d[:, :])
        src_t = const.tile([P, n_sb * 4], i32)
        nc.sync.dma_start(src_t[:], src_d[:, :])
        tar_t = const.tile([P, n_sb * 4], i32)
        nc.sync.dma_start(tar_t[:], tar_d[:, :])

        loop_cm = tc.For_i(0, hw_loop, 1) if hw_loop else nullcontext()
        with loop_cm:
            _emit_body(nc, tc, bass, mybir, FT, f32, n_sb, repeat, x_d, out_d,
                       gpool, tpool, hpool, spool, psT, psH, psS,
                       ident, w1a_t, w1b_t, b1_t, w2_t, b2_t, src_t, tar_t)

    nc.compile()
    return nc


def _emit_body(nc, tc, bass, mybir, FT, f32, n_sb, repeat, x_d, out_d,
               gpool, tpool, hpool, spool, psT, psH, psS,
               ident, w1a_t, w1b_t, b1_t, w2_t, b2_t, src_t, tar_t):
    P = 128
    D = 128
    if True:
        for sb_rep in range(n_sb * repeat):
            sb = sb_rep % n_sb
            xs = gpool.tile([P, 4, D], f32, tag="xs")
            xt = gpool.tile([P, 4, D], f32, tag="xt")
            for c in range(4):
                nc.gpsimd.indirect_dma_start(
                    out=xs[:, c, :],
                    out_offset=None,
                    in_=x_d[:, :],
                    in_offset=bass.IndirectOffsetOnAxis(
                        ap=src_t[:, sb * 4 + c : sb * 4 + c + 1], axis=0
                    ),
                )
                nc.gpsimd.indirect_dma_start(
                    out=xt[:, c, :],
                    out_offset=None,
                    in_=x_d[:, :],
                    in_offset=bass.IndirectOffsetOnAxis(
                        ap=tar_t[:, sb * 4 + c : sb * 4 + c + 1], axis=0
                    ),
                )
            xsT_ps = psT.tile([P, SB], f32, tag="xsT")
            xtT_ps = psT.tile([P, SB], f32, tag="xtT")
            for c in range(4):
                nc.tensor.transpose(
                    out=xsT_ps[:, c * P : (c + 1) * P], in_=xs[:, c, :], identity=ident[:]
                )
                nc.tensor.transpose(
                    out=xtT_ps[:, c * P : (c + 1) * P], in_=xt[:, c, :], identity=ident[:]
                )
            xsT = tpool.tile([P, SB], f32, tag="xsTs")
            xtT = tpool.tile([P, SB], f32, tag="xtTs")
            nc.vector.tensor_copy(xsT[:], xsT_ps[:])
            nc.vector.tensor_copy(xtT[:], xtT_ps[:])

            h_ps = psH.tile([P, SB], f32, tag="h")
            nc.tensor.matmul(h_ps[:], lhsT=w1a_t[:], rhs=xsT[:], start=True, stop=False)
            nc.tensor.matmul(h_ps[:], lhsT=w1b_t[:], rhs=xtT[:], start=False, stop=True)

            hrelu = hpool.tile([P, SB], f32, tag="hrelu")
            nc.scalar.activation(
                out=hrelu[:], in_=h_ps[:], func=FT.Relu, bias=b1_t[:, 0:1]
            )

            s_ps = psS.tile([1, SB], f32, tag="s")
            nc.tensor.matmul(s_ps[:], lhsT=w2_t[:], rhs=hrelu[:], start=True, stop=True)

            s_sb = spool.tile([1, SB], f32, tag="sout")
            nc.scalar.activation(
                out=s_sb[:], in_=s_ps[:], func=FT.Identity, bias=b2_t[0:1, 0:1]
            )
            nc.sync.dma_start(out_d[sb : sb + 1, :], s_sb[0:1, :])


def _permute_idx(idx, n_sb):
    """[n_sb*512] -> [128, n_sb*4] with [p, sb*4+c] = idx[sb*512 + c*128 + p]."""
    return np.ascontiguousarray(
        idx.reshape(n_sb, 4, P).transpose(2, 0, 1).reshape(P, n_sb * 4)
    )


_CACHE = {}


def kernel(**inputs):
    x = np.ascontiguousarray(np.asarray(inputs["x"], dtype=np.float32))
    pos = np.asarray(inputs["pos_edge_index"])
    neg = np.asarray(inputs["neg_edge_index"])
    W1 = np.asarray(inputs["W1"], dtype=np.float32)
    b1 = np.asarray(inputs["b1"], dtype=np.float32)
    W2 = np.asarray(inputs["W2"], dtype=np.float32)
    b2 = np.asarray(inputs["b2"], dtype=np.float32)

    edge = np.concatenate([pos, neg], axis=1).astype(np.int32)  # [2, E_TOTAL]
    src, tar = edge[0], edge[1]
    n_sb = E_PER_CORE // SB

    if "nc" not in _CACHE:
        _CACHE["nc"] = build_nc(n_sb)
    nc = _CACHE["nc"]

    w1a = np.ascontiguousarray(W1[:D, :])
    w1b = np.ascontiguousarray(W1[D:, :])
    b1c = np.ascontiguousarray(b1.reshape(D, 1))
    w2c = np.ascontiguousarray(W2.reshape(D, 1))
    b2c = np.ascontiguousarray(b2.reshape(1, 1))

    in_maps = []
    for c in range(N_CORES):
        lo, hi = c * E_PER_CORE, (c + 1) * E_PER_CORE
        in_maps.append(
            {
                "x": x,
                "src": _permute_idx(src[lo:hi], n_sb),
                "tar": _permute_idx(tar[lo:hi], n_sb),
                "w1a": w1a,
                "w1b": w1b,
                "b1": b1c,
                "w2": w2c,
                "b2": b2c,
            }
        )

    from concourse.bass_utils import run_bass_kernel_spmd

    res = run_bass_kernel_spmd(nc, in_maps, list(range(N_CORES))).results
    out = np.concatenate([res[c]["out"].reshape(-1) for c in range(N_CORES)])
    return out.reshape(E_TOTAL, 1).astype(np.float32)


if __name__ == "__main__":
    # smoke test with random data
    rng = np.random.default_rng(0)
    ins = {
        "x": rng.standard_normal((N_NODES, D), dtype=np.float32),
        "pos_edge_index": rng.integers(0, N_NODES, (2, E_TOTAL // 2)),
        "neg_edge_index": rng.integers(0, N_NODES, (2, E_TOTAL // 2)),
        "W1": rng.standard_normal((2 * D, D), dtype=np.float32) * 0.06,
        "b1": rng.standard_normal(D, dtype=np.float32) * 0.06,
        "W2": rng.standard_normal((D, 1), dtype=np.float32) * 0.09,
        "b2": rng.standard_normal(1, dtype=np.float32) * 0.09,
    }
    out = kernel(**ins)
    h = np.maximum(
        ins["x"][np.concatenate([ins["pos_edge_index"][0], ins["neg_edge_index"][0]])]
        @ ins["W1"][:D]
        + ins["x"][np.concatenate([ins["pos_edge_index"][1], ins["neg_edge_index"][1]])]
        @ ins["W1"][D:]
        + ins["b1"],
        0.0,
    )
    exp = h @ ins["W2"] + ins["b2"]
    err = np.abs(out - exp).max() / max(np.abs(exp).max(), 1e-9)
    print("max rel err:", err)


# revision 15
# speedup vs baseline: 8.0907x; 1.1030x over previous
"""Trainium2 Bass kernel for nn_Decoder (GNN edge decoder / link predictor).

Math (per edge e with endpoints src[e], tar[e]):
    h   = relu(x[src] @ W1[:D] + x[tar] @ W1[D:] + b1)        # [E, D]
    out = h @ W2 + b2                                          # [E, 1]

Strategy (8 NeuronCores, SPMD):
  - Shard the 524288 edges across 8 cores (65536 edges each); replicate x
    and the MLP weights. No collectives.
  - Per core, loop over 128 "superblocks" of 512 edges:
      * indirect-DMA gather x[src] and x[tar] rows (128 rows/partition-major
        tile, 4 blocks of 128 edges)
      * PE-transpose each 128x128 block -> xT layout [feature, edge]
      * hT = W1a.T @ xsT + W1b.T @ xtT   (weights stationary, PSUM accum)
      * relu(hT + b1) fused on ACT (per-partition bias)
      * scores = W2.T @ relu_hT  (PE, M=1) ; + b2 on ACT
      * DMA scores back to DRAM
Host reorders edge indices so each core's output is contiguous and in
original edge order.
"""

import sys
from contextlib import ExitStack

import numpy as np

if "/opt/trn_rl_repo" not in sys.path:
    sys.path.insert(0, "/opt/trn_rl_repo")

N_NODES = 100000
D = 128
E_TOTAL = 524288
N_CORES = 8
E_PER_CORE = E_TOTAL // N_CORES  # 65536
SB = 512  # edges per superblock
P = 128


def build_nc(n_sb=E_PER_CORE // SB, repeat=1, x_external=True):
    """Build the single-core Bass program (SPMD: same program, per-core data).

    repeat>1 re-runs the whole edge workload (same outputs) for steady-state
    HW timing via the delta method; results are identical.
    x_external=False makes x an internal (uninitialized) DRAM tensor so the
    timing harness doesn't ship 51MB/core per call; timing is data-independent.
    """
    return _build_nc_impl(n_sb, repeat, x_external, hw_loop=0)


def _build_nc_impl(n_sb, repeat, x_external, hw_loop, mode="all"):
    import concourse.bacc as bacc
    import concourse.bass as bass
    import concourse.mybir as mybir
    import concourse.tile as tile
    from concourse.masks import make_identity
    from contextlib import ExitStack, nullcontext

    f32 = mybir.dt.float32
    i32 = mybir.dt.int32
    FT = mybir.ActivationFunctionType

    nc = bacc.Bacc("TRN2", target_bir_lowering=False, debug=False)
    if x_external:
        x_d = nc.dram_tensor("x", [N_NODES, D], f32, kind="ExternalInput")
    else:
        x_d = nc.dram_tensor("x", [N_NODES, D], f32)
    # idx layout: [p, sb*4 + c] = node index for edge  sb*512 + c*128 + p
    src_d = nc.dram_tensor("src", [P, n_sb * 4], i32, kind="ExternalInput")
    tar_d = nc.dram_tensor("tar", [P, n_sb * 4], i32, kind="ExternalInput")
    w1a_d = nc.dram_tensor("w1a", [D, D], f32, kind="ExternalInput")
    w1b_d = nc.dram_tensor("w1b", [D, D], f32, kind="ExternalInput")
    b1_d = nc.dram_tensor("b1", [D, 1], f32, kind="ExternalInput")
    w2_d = nc.dram_tensor("w2", [D, 1], f32, kind="ExternalInput")
    b2_d = nc.dram_tensor("b2", [1, 1], f32, kind="ExternalInput")
    out_d = nc.dram_tensor("out", [n_sb, SB], f32, kind="ExternalOutput")

    with tile.TileContext(nc) as tc, ExitStack() as ctx:
        const = ctx.enter_context(tc.tile_pool(name="const", bufs=1))
        gpool = ctx.enter_context(tc.tile_pool(name="gath", bufs=3))
        tpool = ctx.enter_context(tc.tile_pool(name="xT", bufs=3))
        hpool = ctx.enter_context(tc.tile_pool(name="h", bufs=3))
        spool = ctx.enter_context(tc.tile_pool(name="s", bufs=4))
        psT = ctx.enter_context(tc.tile_pool(name="psT", bufs=2, space="PSUM"))
        psH = ctx.enter_context(tc.tile_pool(name="psH", bufs=2, space="PSUM"))
        psS = ctx.enter_context(tc.tile_pool(name="psS", bufs=2, space="PSUM"))

        ident = const.tile([P, P], f32)
        make_identity(nc, ident[:])
        w1a_t = const.tile([D, D], f32)
        nc.sync.dma_start(w1a_t[:], w1a_d[:, :])
        w1b_t = const.tile([D, D], f32)
        nc.sync.dma_start(w1b_t[:], w1b_d[:, :])
        b1_t = const.tile([D, 1], f32)
        nc.sync.dma_start(b1_t[:], b1_d[:, :])
        w2_t = const.tile([D, 1], f32)
        nc.sync.dma_start(w2_t[:], w2_d[:, :])
        b2_t = const.tile([1, 1], f32)
        nc.sync.dma_start(b2_t[:], b2_d[:, :])
        src_t = const.tile([P, n_sb * 4], i32)
        nc.sync.dma_start(src_t[:], src_d[:, :])
        tar_t = const.tile([P, n_sb * 4], i32)
        nc.sync.dma_start(tar_t[:], tar_d[:, :])

        loop_cm = tc.For_i(0, hw_loop, 1) if hw_loop else nullcontext()
        with loop_cm:
            _emit_body(nc, tc, bass, mybir, FT, f32, n_sb, repeat, x_d, out_d,
                       gpool, tpool, hpool, spool, psT, psH, psS,
                       ident, w1a_t, w1b_t, b1_t, w2_t, b2_t, src_t, tar_t, mode)

    nc.compile()
    return nc


def _emit_body(nc, tc, bass, mybir, FT, f32, n_sb, repeat, x_d, out_d,
               gpool, tpool, hpool, spool, psT, psH, psS,
               ident, w1a_t, w1b_t, b1_t, w2_t, b2_t, src_t, tar_t, mode="all"):
    P = 128
    D = 128
    do_gather = mode in ("all", "gather")
    do_compute = mode in ("all", "compute")
    if True:
        for sb_rep in range(n_sb * repeat):
            sb = sb_rep % n_sb
            xs = gpool.tile([P, 4, D], f32, tag="xs")
            xt = gpool.tile([P, 4, D], f32, tag="xt")
            if do_gather:
                for c in range(4):
                    nc.gpsimd.indirect_dma_start(
                        out=xs[:, c, :],
                        out_offset=None,
                        in_=x_d[:, :],
                        in_offset=bass.IndirectOffsetOnAxis(
                            ap=src_t[:, sb * 4 + c : sb * 4 + c + 1], axis=0
                        ),
                    )
                    nc.gpsimd.indirect_dma_start(
                        out=xt[:, c, :],
                        out_offset=None,
                        in_=x_d[:, :],
                        in_offset=bass.IndirectOffsetOnAxis(
                            ap=tar_t[:, sb * 4 + c : sb * 4 + c + 1], axis=0
                        ),
                    )
            if not do_compute:
                continue
            xsT_ps = psT.tile([P, SB], f32, tag="xsT")
            xtT_ps = psT.tile([P, SB], f32, tag="xtT")
            for c in range(4):
                nc.tensor.transpose(
                    out=xsT_ps[:, c * P : (c + 1) * P], in_=xs[:, c, :], identity=ident[:]
                )
                nc.tensor.transpose(
                    out=xtT_ps[:, c * P : (c + 1) * P], in_=xt[:, c, :], identity=ident[:]
                )
            xsT = tpool.tile([P, SB], f32, tag="xsTs")
            xtT = tpool.tile([P, SB], f32, tag="xtTs")
            nc.vector.tensor_copy(xsT[:], xsT_ps[:])
            nc.vector.tensor_copy(xtT[:], xtT_ps[:])

            h_ps = psH.tile([P, SB], f32, tag="h")
            nc.tensor.matmul(h_ps[:], lhsT=w1a_t[:], rhs=xsT[:], start=True, stop=False)
            nc.tensor.matmul(h_ps[:], lhsT=w1b_t[:], rhs=xtT[:], start=False, stop=True)

            hrelu = hpool.tile([P, SB], f32, tag="hrelu")
            nc.scalar.activation(
                out=hrelu[:], in_=h_ps[:], func=FT.Relu, bias=b1_t[:, 0:1]
            )

            s_ps = psS.tile([1, SB], f32, tag="s")
            nc.tensor.matmul(s_ps[:], lhsT=w2_t[:], rhs=hrelu[:], start=True, stop=True)

            s_sb = spool.tile([1, SB], f32, tag="sout")
            nc.scalar.activation(
                out=s_sb[:], in_=s_ps[:], func=FT.Identity, bias=b2_t[0:1, 0:1]
            )
            nc.sync.dma_start(out_d[sb : sb + 1, :], s_sb[0:1, :])


def _permute_idx(idx, n_sb):
    """[n_sb*512] -> [128, n_sb*4] with [p, sb*4+c] = idx[sb*512 + c*128 + p]."""
    return np.ascontiguousarray(
        idx.reshape(n_sb, 4, P).transpose(2, 0, 1).reshape(P, n_sb * 4)
    )


_CACHE = {}


def kernel(**inputs):
    x = np.ascontiguousarray(np.asarray(inputs["x"], dtype=np.float32))
    pos = np.asarray(inputs["pos_edge_index"])
    neg = np.asarray(inputs["neg_edge_index"])
    W1 = np.asarray(inputs["W1"], dtype=np.float32)
    b1 = np.asarray(inputs["b1"], dtype=np.float32)
    W2 = np.asarray(inputs["W2"], dtype=np.float32)
    b2 = np.asarray(inputs["b2"], dtype=np.float32)

    edge = np.concatenate([pos, neg], axis=1).astype(np.int32)  # [2, E_TOTAL]
    src, tar = edge[0], edge[1]
    n_sb = E_PER_CORE // SB

    if "nc" not in _CACHE:
        _CACHE["nc"] = build_nc(n_sb)
    nc = _CACHE["nc"]

    w1a = np.ascontiguousarray(W1[:D, :])
    w1b = np.ascontiguousarray(W1[D:, :])
    b1c = np.ascontiguousarray(b1.reshape(D, 1))
    w2c = np.ascontiguousarray(W2.reshape(D, 1))
    b2c = np.ascontiguousarray(b2.reshape(1, 1))

    in_maps = []
    for c in range(N_CORES):
        lo, hi = c * E_PER_CORE, (c + 1) * E_PER_CORE
        in_maps.append(
            {
                "x": x,
                "src": _permute_idx(src[lo:hi], n_sb),
                "tar": _permute_idx(tar[lo:hi], n_sb),
                "w1a": w1a,
                "w1b": w1b,
                "b1": b1c,
                "w2": w2c,
                "b2": b2c,
            }
        )

    from concourse.bass_utils import run_bass_kernel_spmd

    res = run_bass_kernel_spmd(nc, in_maps, list(range(N_CORES))).results
    out = np.concatenate([res[c]["out"].reshape(-1) for c in range(N_CORES)])
    return out.reshape(E_TOTAL, 1).astype(np.float32)


if __name__ == "__main__":
    # smoke test with random data
    rng = np.random.default_rng(0)
    ins = {
        "x": rng.standard_normal((N_NODES, D), dtype=np.float32),
        "pos_edge_index": rng.integers(0, N_NODES, (2, E_TOTAL // 2)),
        "neg_edge_index": rng.integers(0, N_NODES, (2, E_TOTAL // 2)),
        "W1": rng.standard_normal((2 * D, D), dtype=np.float32) * 0.06,
        "b1": rng.standard_normal(D, dtype=np.float32) * 0.06,
        "W2": rng.standard_normal((D, 1), dtype=np.float32) * 0.09,
        "b2": rng.standard_normal(1, dtype=np.float32) * 0.09,
    }
    out = kernel(**ins)
    h = np.maximum(
        ins["x"][np.concatenate([ins["pos_edge_index"][0], ins["neg_edge_index"][0]])]
        @ ins["W1"][:D]
        + ins["x"][np.concatenate([ins["pos_edge_index"][1], ins["neg_edge_index"][1]])]
        @ ins["W1"][D:]
        + ins["b1"],
        0.0,
    )
    exp = h @ ins["W2"] + ins["b2"]
    err = np.abs(out - exp).max() / max(np.abs(exp).max(), 1e-9)
    print("max rel err:", err)


# revision 18
# speedup vs baseline: 8.7604x; 1.0828x over previous
"""Trainium2 Bass kernel for nn_Decoder (GNN edge decoder / link predictor).

Math (per edge e with endpoints src[e], tar[e]):
    h   = relu(x[src] @ W1[:D] + x[tar] @ W1[D:] + b1)        # [E, D]
    out = h @ W2 + b2                                          # [E, 1]

Strategy (8 NeuronCores, SPMD):
  - Shard the 524288 edges across 8 cores (65536 edges each); replicate x
    and the MLP weights on every core. No collectives.
  - Random row gather is the bottleneck. SWDGE has ~1us fixed cost per DMA
    instruction, so per-128-row indirect DMAs are too slow. Instead use the
    dma_gather ucode (InstDMAGatherAnt) with 2048 rows per instruction.
    dma_gather indices are int16 (max 32767), so the host buckets each
    core's edges by the (src_window, tar_window) pair, where a window is a
    32768-row slice of x. Each bucket has a static capacity (so the program
    is input-independent); pad slots gather row 0 and are discarded.
  - Per 512-edge superblock: PE-transpose 128x128 blocks -> [feature, edge]
    layout, hT = W1a.T@xsT + W1b.T@xtT (weights stationary, PSUM accum),
    relu(hT + b1) on ACT (per-partition bias), scores = W2.T @ relu_hT on
    PE, + b2 on ACT, DMA out.
  - Host maps device slots back to original edge order at the end.
"""

import sys
from contextlib import ExitStack, nullcontext

import numpy as np

if "/opt/trn_rl_repo" not in sys.path:
    sys.path.insert(0, "/opt/trn_rl_repo")

N_NODES = 100000
D = 128
E_TOTAL = 524288
N_CORES = 8
E_PER_CORE = E_TOTAL // N_CORES  # 65536
SB = 512  # edges per superblock
P = 128
WIN = 32768  # index window (int16 range)
N_WIN = 4  # ceil(100000 / 32768)
GIDX = 1024  # rows per dma_gather instruction
WLEN = [WIN, WIN, WIN, N_NODES - 3 * WIN]  # rows per window


def default_caps(n_edges=E_PER_CORE):
    """Static per-bucket slot capacities (multiples of SB), sized at
    mean + ~6 sigma for uniform random endpoints."""
    pw = np.array([WLEN[0], WLEN[1], WLEN[2], WLEN[3]], np.float64) / N_NODES
    caps = []
    for ws in range(N_WIN):
        for wt in range(N_WIN):
            pb = pw[ws] * pw[wt]
            mean = n_edges * pb
            std = np.sqrt(n_edges * pb * (1 - pb))
            need = mean + 6.0 * std + 8
            caps.append(max(SB, int(np.ceil(need / SB)) * SB))
    return tuple(caps)


def gather_split(cap):
    """Split a bucket capacity into dma_gather instruction sizes."""
    out = []
    while cap > 0:
        g = min(GIDX, cap)
        out.append(g)
        cap -= g
    return out


def build_nc(caps, repeat=1, x_external=True, hw_loop=0, mode="all"):
    import concourse.bacc as bacc
    import concourse.bass as bass
    import concourse.mybir as mybir
    import concourse.tile as tile
    from concourse.masks import make_identity

    f32 = mybir.dt.float32
    i16 = mybir.dt.int16
    FT = mybir.ActivationFunctionType

    S = int(sum(caps))
    n_sb = S // SB

    nc = bacc.Bacc("TRN2", target_bir_lowering=False, debug=False)
    if x_external:
        x_d = nc.dram_tensor("x", [N_NODES, D], f32, kind="ExternalInput")
    else:
        x_d = nc.dram_tensor("x", [N_NODES, D], f32)
    # wrapped int16 index tables: [p, j] = local_idx of slot (j*16 + p%16)
    src_d = nc.dram_tensor("src", [P, S // 16], i16, kind="ExternalInput")
    tar_d = nc.dram_tensor("tar", [P, S // 16], i16, kind="ExternalInput")
    w1a_d = nc.dram_tensor("w1a", [D, D], f32, kind="ExternalInput")
    w1b_d = nc.dram_tensor("w1b", [D, D], f32, kind="ExternalInput")
    b1_d = nc.dram_tensor("b1", [D, 1], f32, kind="ExternalInput")
    w2_d = nc.dram_tensor("w2", [D, 1], f32, kind="ExternalInput")
    b2_d = nc.dram_tensor("b2", [1, 1], f32, kind="ExternalInput")
    out_d = nc.dram_tensor("out", [n_sb, SB], f32, kind="ExternalOutput")

    do_gather = mode in ("all", "gather")
    do_compute = mode in ("all", "compute")

    with tile.TileContext(nc) as tc, ExitStack() as ctx:
        const = ctx.enter_context(tc.tile_pool(name="const", bufs=1))
        gpool = ctx.enter_context(tc.tile_pool(name="gath", bufs=3))
        tpool = ctx.enter_context(tc.tile_pool(name="xT", bufs=3))
        hpool = ctx.enter_context(tc.tile_pool(name="h", bufs=3))
        spool = ctx.enter_context(tc.tile_pool(name="s", bufs=4))
        psT = ctx.enter_context(tc.tile_pool(name="psT", bufs=2, space="PSUM"))
        psH = ctx.enter_context(tc.tile_pool(name="psH", bufs=2, space="PSUM"))
        psS = ctx.enter_context(tc.tile_pool(name="psS", bufs=2, space="PSUM"))

        ident = const.tile([P, P], f32)
        make_identity(nc, ident[:])
        w1a_t = const.tile([D, D], f32)
        nc.sync.dma_start(w1a_t[:], w1a_d[:, :])
        w1b_t = const.tile([D, D], f32)
        nc.sync.dma_start(w1b_t[:], w1b_d[:, :])
        b1_t = const.tile([D, 1], f32)
        nc.sync.dma_start(b1_t[:], b1_d[:, :])
        w2_t = const.tile([D, 1], f32)
        nc.sync.dma_start(w2_t[:], w2_d[:, :])
        b2_t = const.tile([1, 1], f32)
        nc.sync.dma_start(b2_t[:], b2_d[:, :])
        src_t = const.tile([P, S // 16], i16)
        nc.sync.dma_start(src_t[:], src_d[:, :])
        tar_t = const.tile([P, S // 16], i16)
        nc.sync.dma_start(tar_t[:], tar_d[:, :])

        x_win = [x_d[w * WIN : w * WIN + WLEN[w], :] for w in range(N_WIN)]

        def body():
            sb_global = 0
            slot_off = 0
            for ws in range(N_WIN):
                for wt in range(N_WIN):
                    cap = caps[ws * N_WIN + wt]
                    for g in gather_split(cap):
                        C = g // P
                        xs_g = gpool.tile([P, C, D], f32, tag="xs")
                        xt_g = gpool.tile([P, C, D], f32, tag="xt")
                        if do_gather:
                            nc.gpsimd.dma_gather(
                                xs_g[:, :, :],
                                x_win[ws],
                                src_t[:, slot_off // 16 : (slot_off + g) // 16],
                                g,
                                g,
                                D,
                            )
                            nc.gpsimd.dma_gather(
                                xt_g[:, :, :],
                                x_win[wt],
                                tar_t[:, slot_off // 16 : (slot_off + g) // 16],
                                g,
                                g,
                                D,
                            )
                        if do_compute:
                            for s in range(g // SB):
                                xsT_ps = psT.tile([P, SB], f32, tag="xsT")
                                xtT_ps = psT.tile([P, SB], f32, tag="xtT")
                                for c in range(4):
                                    nc.tensor.transpose(
                                        out=xsT_ps[:, c * P : (c + 1) * P],
                                        in_=xs_g[:, 4 * s + c, :],
                                        identity=ident[:],
                                    )
                                    nc.tensor.transpose(
                                        out=xtT_ps[:, c * P : (c + 1) * P],
                                        in_=xt_g[:, 4 * s + c, :],
                                        identity=ident[:],
                                    )
                                xsT = tpool.tile([P, SB], f32, tag="xsTs")
                                xtT = tpool.tile([P, SB], f32, tag="xtTs")
                                nc.vector.tensor_copy(xsT[:], xsT_ps[:])
                                nc.vector.tensor_copy(xtT[:], xtT_ps[:])

                                h_ps = psH.tile([P, SB], f32, tag="h")
                                nc.tensor.matmul(
                                    h_ps[:], lhsT=w1a_t[:], rhs=xsT[:],
                                    start=True, stop=False,
                                )
                                nc.tensor.matmul(
                                    h_ps[:], lhsT=w1b_t[:], rhs=xtT[:],
                                    start=False, stop=True,
                                )
                                hrelu = hpool.tile([P, SB], f32, tag="hrelu")
                                nc.scalar.activation(
                                    out=hrelu[:], in_=h_ps[:], func=FT.Relu,
                                    bias=b1_t[:, 0:1],
                                )
                                s_ps = psS.tile([1, SB], f32, tag="s")
                                nc.tensor.matmul(
                                    s_ps[:], lhsT=w2_t[:], rhs=hrelu[:],
                                    start=True, stop=True,
                                )
                                s_sb = spool.tile([1, SB], f32, tag="sout")
                                nc.scalar.activation(
                                    out=s_sb[:], in_=s_ps[:], func=FT.Identity,
                                    bias=b2_t[0:1, 0:1],
                                )
                                nc.sync.dma_start(
                                    out_d[sb_global : sb_global + 1, :], s_sb[0:1, :]
                                )
                                sb_global += 1
                        slot_off += g

        loop_cm = tc.For_i(0, hw_loop, 1) if hw_loop else nullcontext()
        with loop_cm:
            for _ in range(repeat):
                body()

    nc.compile()
    return nc


def prep_core(src, tar, caps):
    """Bucket one core's edges; returns wrapped int16 idx tables and the
    slot index of each edge (or None on capacity overflow)."""
    n_edges = len(src)
    S = int(sum(caps))
    ws = src >> 15
    wt = tar >> 15
    b = ws * N_WIN + wt
    sizes = np.bincount(b, minlength=16)
    if np.any(sizes > np.asarray(caps)):
        return None
    order = np.argsort(b, kind="stable")
    base = np.concatenate([[0], np.cumsum(caps)]).astype(np.int64)
    cum = np.concatenate([[0], np.cumsum(sizes)]).astype(np.int64)
    vsrc = np.zeros(S, np.int16)
    vtar = np.zeros(S, np.int16)
    slot_of_edge = np.empty(n_edges, np.int64)
    for bb in range(16):
        e = order[cum[bb] : cum[bb + 1]]
        slots = base[bb] + np.arange(len(e))
        slot_of_edge[e] = slots
        vsrc[slots] = (src[e] & 32767).astype(np.int16)
        vtar[slots] = (tar[e] & 32767).astype(np.int16)

    def wrap(v):
        t = v.reshape(S // 16, 16).T  # [16, S/16]
        return np.ascontiguousarray(np.tile(t, (P // 16, 1)))

    return wrap(vsrc), wrap(vtar), slot_of_edge


_CACHE = {}


def _get_nc(caps):
    key = ("nc", caps)
    if key not in _CACHE:
        _CACHE[key] = build_nc(caps)
    return _CACHE[key]


def kernel(**inputs):
    x = np.ascontiguousarray(np.asarray(inputs["x"], dtype=np.float32))
    pos = np.asarray(inputs["pos_edge_index"])
    neg = np.asarray(inputs["neg_edge_index"])
    W1 = np.asarray(inputs["W1"], dtype=np.float32)
    b1 = np.asarray(inputs["b1"], dtype=np.float32)
    W2 = np.asarray(inputs["W2"], dtype=np.float32)
    b2 = np.asarray(inputs["b2"], dtype=np.float32)

    edge = np.concatenate([pos, neg], axis=1).astype(np.int64)  # [2, E_TOTAL]
    src, tar = edge[0], edge[1]

    caps = default_caps()
    preps = []
    for c in range(N_CORES):
        lo, hi = c * E_PER_CORE, (c + 1) * E_PER_CORE
        pr = prep_core(src[lo:hi], tar[lo:hi], caps)
        if pr is None:
            # capacity overflow (shouldn't happen for uniform random inputs):
            # rebuild with actual sizes + slack
            sizes = np.zeros(16, np.int64)
            for cc in range(N_CORES):
                l2, h2 = cc * E_PER_CORE, (cc + 1) * E_PER_CORE
                bb = (src[l2:h2] >> 15) * N_WIN + (tar[l2:h2] >> 15)
                sizes = np.maximum(sizes, np.bincount(bb, minlength=16))
            caps = tuple(
                int(np.ceil((s + 256) / SB)) * SB for s in sizes
            )
            preps = []
            for cc in range(N_CORES):
                l2, h2 = cc * E_PER_CORE, (cc + 1) * E_PER_CORE
                preps.append(prep_core(src[l2:h2], tar[l2:h2], caps))
            break
        preps.append(pr)

    nc = _get_nc(caps)

    w1a = np.ascontiguousarray(W1[:D, :])
    w1b = np.ascontiguousarray(W1[D:, :])
    b1c = np.ascontiguousarray(b1.reshape(D, 1))
    w2c = np.ascontiguousarray(W2.reshape(D, 1))
    b2c = np.ascontiguousarray(b2.reshape(1, 1))

    in_maps = []
    for c in range(N_CORES):
        vsrc, vtar, _ = preps[c]
        in_maps.append(
            {
                "x": x,
                "src": vsrc,
                "tar": vtar,
                "w1a": w1a,
                "w1b": w1b,
                "b1": b1c,
                "w2": w2c,
                "b2": b2c,
            }
        )

    from concourse.bass_utils import run_bass_kernel_spmd

    _CACHE["in_maps"] = in_maps
    _CACHE["caps"] = caps
    res = run_bass_kernel_spmd(nc, in_maps, list(range(N_CORES))).results
    out = np.empty((E_TOTAL,), np.float32)
    for c in range(N_CORES):
        flat = res[c]["out"].reshape(-1)
        lo = c * E_PER_CORE
        out[lo : lo + E_PER_CORE] = flat[preps[c][2]]
    return out.reshape(E_TOTAL, 1).astype(np.float32)


if __name__ == "__main__":
    rng = np.random.default_rng(0)
    ins = {
        "x": rng.standard_normal((N_NODES, D), dtype=np.float32),
        "pos_edge_index": rng.integers(0, N_NODES, (2, E_TOTAL // 2)),
        "neg_edge_index": rng.integers(0, N_NODES, (2, E_TOTAL // 2)),
        "W1": rng.standard_normal((2 * D, D), dtype=np.float32) * 0.06,
        "b1": rng.standard_normal(D, dtype=np.float32) * 0.06,
        "W2": rng.standard_normal((D, 1), dtype=np.float32) * 0.09,
        "b2": rng.standard_normal(1, dtype=np.float32) * 0.09,
    }
    out = kernel(**ins)
    s = np.concatenate([ins["pos_edge_index"][0], ins["neg_edge_index"][0]])
    t = np.concatenate([ins["pos_edge_index"][1], ins["neg_edge_index"][1]])
    h = np.maximum(ins["x"][s] @ ins["W1"][:D] + ins["x"][t] @ ins["W1"][D:] + ins["b1"], 0.0)
    exp = h @ ins["W2"] + ins["b2"]
    err = np.abs(out - exp).max() / max(np.abs(exp).max(), 1e-9)
    print("max rel err:", err)


# revision 20
# speedup vs baseline: 17.9303x; 2.0468x over previous
"""Trainium2 Bass kernel for nn_Decoder (GNN edge decoder / link predictor).

Math (per edge e with endpoints src[e], tar[e]):
    h   = relu(x[src] @ W1[:D] + x[tar] @ W1[D:] + b1)        # [E, D]
    out = h @ W2 + b2                                          # [E, 1]

Strategy (8 NeuronCores, SPMD):
  - Shard the 524288 edges across 8 cores (65536 edges each); replicate x
    and the MLP weights on every core. No collectives.
  - Random row gather is the bottleneck. SWDGE has ~1us fixed cost per DMA
    instruction, so per-128-row indirect DMAs are too slow. Instead use the
    dma_gather ucode (InstDMAGatherAnt) with 2048 rows per instruction.
    dma_gather indices are int16 (max 32767), so the host buckets each
    core's edges by the (src_window, tar_window) pair, where a window is a
    32768-row slice of x. Each bucket has a static capacity (so the program
    is input-independent); pad slots gather row 0 and are discarded.
  - Per 512-edge superblock: PE-transpose 128x128 blocks -> [feature, edge]
    layout, hT = W1a.T@xsT + W1b.T@xtT (weights stationary, PSUM accum),
    relu(hT + b1) on ACT (per-partition bias), scores = W2.T @ relu_hT on
    PE, + b2 on ACT, DMA out.
  - Host maps device slots back to original edge order at the end.
"""

import sys
from contextlib import ExitStack, nullcontext

import numpy as np

if "/opt/trn_rl_repo" not in sys.path:
    sys.path.insert(0, "/opt/trn_rl_repo")

N_NODES = 100000
D = 128
E_TOTAL = 524288
N_CORES = 8
E_PER_CORE = E_TOTAL // N_CORES  # 65536
SB = 512  # edges per superblock
P = 128
WIN = 32768  # index window (int16 range)
N_WIN = 4  # ceil(100000 / 32768)
GIDX = 1024  # rows per dma_gather instruction
WLEN = [WIN, WIN, WIN, N_NODES - 3 * WIN]  # rows per window


def default_caps(n_edges=E_PER_CORE):
    """Static per-bucket slot capacities (multiples of SB), sized at
    mean + ~6 sigma for uniform random endpoints."""
    pw = np.array([WLEN[0], WLEN[1], WLEN[2], WLEN[3]], np.float64) / N_NODES
    caps = []
    for ws in range(N_WIN):
        for wt in range(N_WIN):
            pb = pw[ws] * pw[wt]
            mean = n_edges * pb
            std = np.sqrt(n_edges * pb * (1 - pb))
            need = mean + 6.0 * std + 8
            caps.append(max(SB, int(np.ceil(need / SB)) * SB))
    return tuple(caps)


def gather_split(cap):
    """Split a bucket capacity into dma_gather instruction sizes."""
    out = []
    while cap > 0:
        g = min(GIDX, cap)
        out.append(g)
        cap -= g
    return out


def build_nc(caps, repeat=1, x_external=True, hw_loop=0, mode="all"):
    import concourse.bacc as bacc
    import concourse.bass as bass
    import concourse.mybir as mybir
    import concourse.tile as tile
    from concourse.masks import make_identity

    f32 = mybir.dt.float32
    i16 = mybir.dt.int16
    FT = mybir.ActivationFunctionType

    S = int(sum(caps))
    n_sb = S // SB

    nc = bacc.Bacc("TRN2", target_bir_lowering=False, debug=False, num_swdge_queues=4)
    if x_external:
        x_d = nc.dram_tensor("x", [N_NODES, D], f32, kind="ExternalInput")
    else:
        x_d = nc.dram_tensor("x", [N_NODES, D], f32)
    # wrapped int16 index tables: [p, j] = local_idx of slot (j*16 + p%16)
    src_d = nc.dram_tensor("src", [P, S // 16], i16, kind="ExternalInput")
    tar_d = nc.dram_tensor("tar", [P, S // 16], i16, kind="ExternalInput")
    w1a_d = nc.dram_tensor("w1a", [D, D], f32, kind="ExternalInput")
    w1b_d = nc.dram_tensor("w1b", [D, D], f32, kind="ExternalInput")
    b1_d = nc.dram_tensor("b1", [D, 1], f32, kind="ExternalInput")
    w2_d = nc.dram_tensor("w2", [D, 1], f32, kind="ExternalInput")
    b2_d = nc.dram_tensor("b2", [1, 1], f32, kind="ExternalInput")
    out_d = nc.dram_tensor("out", [n_sb, SB], f32, kind="ExternalOutput")

    do_gather = mode in ("all", "gather")
    do_compute = mode in ("all", "compute")

    with tile.TileContext(nc) as tc, ExitStack() as ctx:
        const = ctx.enter_context(tc.tile_pool(name="const", bufs=1))
        gpool = ctx.enter_context(tc.tile_pool(name="gath", bufs=3))
        tpool = ctx.enter_context(tc.tile_pool(name="xT", bufs=3))
        hpool = ctx.enter_context(tc.tile_pool(name="h", bufs=3))
        spool = ctx.enter_context(tc.tile_pool(name="s", bufs=4))
        psT = ctx.enter_context(tc.tile_pool(name="psT", bufs=2, space="PSUM"))
        psH = ctx.enter_context(tc.tile_pool(name="psH", bufs=2, space="PSUM"))
        psS = ctx.enter_context(tc.tile_pool(name="psS", bufs=2, space="PSUM"))

        ident = const.tile([P, P], f32)
        make_identity(nc, ident[:])
        w1a_t = const.tile([D, D], f32)
        nc.sync.dma_start(w1a_t[:], w1a_d[:, :])
        w1b_t = const.tile([D, D], f32)
        nc.sync.dma_start(w1b_t[:], w1b_d[:, :])
        b1_t = const.tile([D, 1], f32)
        nc.sync.dma_start(b1_t[:], b1_d[:, :])
        w2_t = const.tile([D, 1], f32)
        nc.sync.dma_start(w2_t[:], w2_d[:, :])
        b2_t = const.tile([1, 1], f32)
        nc.sync.dma_start(b2_t[:], b2_d[:, :])
        src_t = const.tile([P, S // 16], i16)
        nc.sync.dma_start(src_t[:], src_d[:, :])
        tar_t = const.tile([P, S // 16], i16)
        nc.sync.dma_start(tar_t[:], tar_d[:, :])

        x_win = [x_d[w * WIN : w * WIN + WLEN[w], :] for w in range(N_WIN)]

        def body():
            sb_global = 0
            slot_off = 0
            qrr = [0]
            for ws in range(N_WIN):
                for wt in range(N_WIN):
                    cap = caps[ws * N_WIN + wt]
                    for g in gather_split(cap):
                        C = g // P
                        xs_g = gpool.tile([P, C, D], f32, tag="xs")
                        xt_g = gpool.tile([P, C, D], f32, tag="xt")
                        if do_gather:
                            nc.gpsimd.dma_gather(
                                xs_g[:, :, :],
                                x_win[ws],
                                src_t[:, slot_off // 16 : (slot_off + g) // 16],
                                g,
                                g,
                                D,
                                queue_num=qrr[0] % 4,
                            )
                            qrr[0] += 1
                            nc.gpsimd.dma_gather(
                                xt_g[:, :, :],
                                x_win[wt],
                                tar_t[:, slot_off // 16 : (slot_off + g) // 16],
                                g,
                                g,
                                D,
                                queue_num=qrr[0] % 4,
                            )
                            qrr[0] += 1
                        if do_compute:
                            for s in range(g // SB):
                                xsT_ps = psT.tile([P, SB], f32, tag="xsT")
                                xtT_ps = psT.tile([P, SB], f32, tag="xtT")
                                for c in range(4):
                                    nc.tensor.transpose(
                                        out=xsT_ps[:, c * P : (c + 1) * P],
                                        in_=xs_g[:, 4 * s + c, :],
                                        identity=ident[:],
                                    )
                                    nc.tensor.transpose(
                                        out=xtT_ps[:, c * P : (c + 1) * P],
                                        in_=xt_g[:, 4 * s + c, :],
                                        identity=ident[:],
                                    )
                                xsT = tpool.tile([P, SB], f32, tag="xsTs")
                                xtT = tpool.tile([P, SB], f32, tag="xtTs")
                                nc.vector.tensor_copy(xsT[:], xsT_ps[:])
                                nc.vector.tensor_copy(xtT[:], xtT_ps[:])

                                h_ps = psH.tile([P, SB], f32, tag="h")
                                nc.tensor.matmul(
                                    h_ps[:], lhsT=w1a_t[:], rhs=xsT[:],
                                    start=True, stop=False,
                                )
                                nc.tensor.matmul(
                                    h_ps[:], lhsT=w1b_t[:], rhs=xtT[:],
                                    start=False, stop=True,
                                )
                                hrelu = hpool.tile([P, SB], f32, tag="hrelu")
                                nc.scalar.activation(
                                    out=hrelu[:], in_=h_ps[:], func=FT.Relu,
                                    bias=b1_t[:, 0:1],
                                )
                                s_ps = psS.tile([1, SB], f32, tag="s")
                                nc.tensor.matmul(
                                    s_ps[:], lhsT=w2_t[:], rhs=hrelu[:],
                                    start=True, stop=True,
                                )
                                s_sb = spool.tile([1, SB], f32, tag="sout")
                                nc.scalar.activation(
                                    out=s_sb[:], in_=s_ps[:], func=FT.Identity,
                                    bias=b2_t[0:1, 0:1],
                                )
                                nc.sync.dma_start(
                                    out_d[sb_global : sb_global + 1, :], s_sb[0:1, :]
                                )
                                sb_global += 1
                        slot_off += g

        loop_cm = tc.For_i(0, hw_loop, 1) if hw_loop else nullcontext()
        with loop_cm:
            for _ in range(repeat):
                body()

    # Tile assigns Pool DMAs to DMASW sem lanes round-robin in *scheduled*
    # order; a DMA semaphore may only be used by one SWDGE queue. Rewrite each
    # gather's queue_num to follow its assigned lane so sem<->queue stays
    # consistent (and the 4 ucode queues are load balanced).
    from concourse.tile_scheduler import PROC_NAME_TO_IDX

    lane_of = {PROC_NAME_TO_IDX[f"DMASW{k}"]: k for k in range(8)}
    for f in nc.m.functions:
        for blk in f.blocks:
            for inst in blk.instructions:
                if isinstance(inst, mybir.InstDMAGatherAnt):
                    inst.queue_num = lane_of[inst.bass_scheduled_proc] % 4

    nc.compile()
    return nc


def prep_core(src, tar, caps):
    """Bucket one core's edges; returns wrapped int16 idx tables and the
    slot index of each edge (or None on capacity overflow)."""
    n_edges = len(src)
    S = int(sum(caps))
    ws = src >> 15
    wt = tar >> 15
    b = ws * N_WIN + wt
    sizes = np.bincount(b, minlength=16)
    if np.any(sizes > np.asarray(caps)):
        return None
    order = np.argsort(b, kind="stable")
    base = np.concatenate([[0], np.cumsum(caps)]).astype(np.int64)
    cum = np.concatenate([[0], np.cumsum(sizes)]).astype(np.int64)
    vsrc = np.zeros(S, np.int16)
    vtar = np.zeros(S, np.int16)
    slot_of_edge = np.empty(n_edges, np.int64)
    for bb in range(16):
        e = order[cum[bb] : cum[bb + 1]]
        slots = base[bb] + np.arange(len(e))
        slot_of_edge[e] = slots
        vsrc[slots] = (src[e] & 32767).astype(np.int16)
        vtar[slots] = (tar[e] & 32767).astype(np.int16)

    def wrap(v):
        t = v.reshape(S // 16, 16).T  # [16, S/16]
        return np.ascontiguousarray(np.tile(t, (P // 16, 1)))

    return wrap(vsrc), wrap(vtar), slot_of_edge


_CACHE = {}


def _get_nc(caps):
    key = ("nc", caps)
    if key not in _CACHE:
        _CACHE[key] = build_nc(caps)
    return _CACHE[key]


def kernel(**inputs):
    x = np.ascontiguousarray(np.asarray(inputs["x"], dtype=np.float32))
    pos = np.asarray(inputs["pos_edge_index"])
    neg = np.asarray(inputs["neg_edge_index"])
    W1 = np.asarray(inputs["W1"], dtype=np.float32)
    b1 = np.asarray(inputs["b1"], dtype=np.float32)
    W2 = np.asarray(inputs["W2"], dtype=np.float32)
    b2 = np.asarray(inputs["b2"], dtype=np.float32)

    edge = np.concatenate([pos, neg], axis=1).astype(np.int64)  # [2, E_TOTAL]
    src, tar = edge[0], edge[1]

    caps = default_caps()
    preps = []
    for c in range(N_CORES):
        lo, hi = c * E_PER_CORE, (c + 1) * E_PER_CORE
        pr = prep_core(src[lo:hi], tar[lo:hi], caps)
        if pr is None:
            # capacity overflow (shouldn't happen for uniform random inputs):
            # rebuild with actual sizes + slack
            sizes = np.zeros(16, np.int64)
            for cc in range(N_CORES):
                l2, h2 = cc * E_PER_CORE, (cc + 1) * E_PER_CORE
                bb = (src[l2:h2] >> 15) * N_WIN + (tar[l2:h2] >> 15)
                sizes = np.maximum(sizes, np.bincount(bb, minlength=16))
            caps = tuple(
                int(np.ceil((s + 256) / SB)) * SB for s in sizes
            )
            preps = []
            for cc in range(N_CORES):
                l2, h2 = cc * E_PER_CORE, (cc + 1) * E_PER_CORE
                preps.append(prep_core(src[l2:h2], tar[l2:h2], caps))
            break
        preps.append(pr)

    nc = _get_nc(caps)

    w1a = np.ascontiguousarray(W1[:D, :])
    w1b = np.ascontiguousarray(W1[D:, :])
    b1c = np.ascontiguousarray(b1.reshape(D, 1))
    w2c = np.ascontiguousarray(W2.reshape(D, 1))
    b2c = np.ascontiguousarray(b2.reshape(1, 1))

    in_maps = []
    for c in range(N_CORES):
        vsrc, vtar, _ = preps[c]
        in_maps.append(
            {
                "x": x,
                "src": vsrc,
                "tar": vtar,
                "w1a": w1a,
                "w1b": w1b,
                "b1": b1c,
                "w2": w2c,
                "b2": b2c,
            }
        )

    from concourse.bass_utils import run_bass_kernel_spmd

    _CACHE["in_maps"] = in_maps
    _CACHE["caps"] = caps
    res = run_bass_kernel_spmd(nc, in_maps, list(range(N_CORES))).results
    out = np.empty((E_TOTAL,), np.float32)
    for c in range(N_CORES):
        flat = res[c]["out"].reshape(-1)
        lo = c * E_PER_CORE
        out[lo : lo + E_PER_CORE] = flat[preps[c][2]]
    return out.reshape(E_TOTAL, 1).astype(np.float32)


if __name__ == "__main__":
    rng = np.random.default_rng(0)
    ins = {
        "x": rng.standard_normal((N_NODES, D), dtype=np.float32),
        "pos_edge_index": rng.integers(0, N_NODES, (2, E_TOTAL // 2)),
        "neg_edge_index": rng.integers(0, N_NODES, (2, E_TOTAL // 2)),
        "W1": rng.standard_normal((2 * D, D), dtype=np.float32) * 0.06,
        "b1": rng.standard_normal(D, dtype=np.float32) * 0.06,
        "W2": rng.standard_normal((D, 1), dtype=np.float32) * 0.09,
        "b2": rng.standard_normal(1, dtype=np.float32) * 0.09,
    }
    out = kernel(**ins)
    s = np.concatenate([ins["pos_edge_index"][0], ins["neg_edge_index"][0]])
    t = np.concatenate([ins["pos_edge_index"][1], ins["neg_edge_index"][1]])
    h = np.maximum(ins["x"][s] @ ins["W1"][:D] + ins["x"][t] @ ins["W1"][D:] + ins["b1"], 0.0)
    exp = h @ ins["W2"] + ins["b2"]
    err = np.abs(out - exp).max() / max(np.abs(exp).max(), 1e-9)
    print("max rel err:", err)


# revision 21
# speedup vs baseline: 19.2640x; 1.0744x over previous
"""Trainium2 Bass kernel for nn_Decoder (GNN edge decoder / link predictor).

Math (per edge e with endpoints src[e], tar[e]):
    h   = relu(x[src] @ W1[:D] + x[tar] @ W1[D:] + b1)        # [E, D]
    out = h @ W2 + b2                                          # [E, 1]

Strategy (8 NeuronCores, SPMD):
  - Shard the 524288 edges across 8 cores (65536 edges each); replicate x
    and the MLP weights on every core. No collectives.
  - Random row gather is the bottleneck. SWDGE has ~1us fixed cost per DMA
    instruction, so per-128-row indirect DMAs are too slow. Instead use the
    dma_gather ucode (InstDMAGatherAnt) with 2048 rows per instruction.
    dma_gather indices are int16 (max 32767), so the host buckets each
    core's edges by the (src_window, tar_window) pair, where a window is a
    32768-row slice of x. Each bucket has a static capacity (so the program
    is input-independent); pad slots gather row 0 and are discarded.
  - Per 512-edge superblock: PE-transpose 128x128 blocks -> [feature, edge]
    layout, hT = W1a.T@xsT + W1b.T@xtT (weights stationary, PSUM accum),
    relu(hT + b1) on ACT (per-partition bias), scores = W2.T @ relu_hT on
    PE, + b2 on ACT, DMA out.
  - Host maps device slots back to original edge order at the end.
"""

import sys
from contextlib import ExitStack, nullcontext

import numpy as np

if "/opt/trn_rl_repo" not in sys.path:
    sys.path.insert(0, "/opt/trn_rl_repo")

N_NODES = 100000
D = 128
E_TOTAL = 524288
N_CORES = 8
E_PER_CORE = E_TOTAL // N_CORES  # 65536
SB = 512  # edges per superblock
P = 128
WIN = 32768  # index window (int16 range)
N_WIN = 4  # ceil(100000 / 32768)
GIDX = 2048  # rows per dma_gather instruction (single_packet=False)
WLEN = [WIN, WIN, WIN, N_NODES - 3 * WIN]  # rows per window


def default_caps(n_edges=E_PER_CORE):
    """Static per-bucket slot capacities (multiples of SB), sized at
    mean + ~6 sigma for uniform random endpoints."""
    pw = np.array([WLEN[0], WLEN[1], WLEN[2], WLEN[3]], np.float64) / N_NODES
    caps = []
    for ws in range(N_WIN):
        for wt in range(N_WIN):
            pb = pw[ws] * pw[wt]
            mean = n_edges * pb
            std = np.sqrt(n_edges * pb * (1 - pb))
            need = mean + 6.0 * std + 8
            caps.append(max(SB, int(np.ceil(need / SB)) * SB))
    return tuple(caps)


def gather_split(cap):
    """Split a bucket capacity into dma_gather instruction sizes."""
    out = []
    while cap > 0:
        g = min(GIDX, cap)
        out.append(g)
        cap -= g
    return out


def build_nc(caps, repeat=1, x_external=True, hw_loop=0, mode="all"):
    import concourse.bacc as bacc
    import concourse.bass as bass
    import concourse.mybir as mybir
    import concourse.tile as tile
    from concourse.masks import make_identity

    f32 = mybir.dt.float32
    i16 = mybir.dt.int16
    FT = mybir.ActivationFunctionType

    S = int(sum(caps))
    n_sb = S // SB

    nc = bacc.Bacc("TRN2", target_bir_lowering=False, debug=False, num_swdge_queues=4)
    if x_external:
        x_d = nc.dram_tensor("x", [N_NODES, D], f32, kind="ExternalInput")
    else:
        x_d = nc.dram_tensor("x", [N_NODES, D], f32)
    # wrapped int16 index tables: [p, j] = local_idx of slot (j*16 + p%16)
    src_d = nc.dram_tensor("src", [P, S // 16], i16, kind="ExternalInput")
    tar_d = nc.dram_tensor("tar", [P, S // 16], i16, kind="ExternalInput")
    w1a_d = nc.dram_tensor("w1a", [D, D], f32, kind="ExternalInput")
    w1b_d = nc.dram_tensor("w1b", [D, D], f32, kind="ExternalInput")
    b1_d = nc.dram_tensor("b1", [D, 1], f32, kind="ExternalInput")
    w2_d = nc.dram_tensor("w2", [D, 1], f32, kind="ExternalInput")
    b2_d = nc.dram_tensor("b2", [1, 1], f32, kind="ExternalInput")
    out_d = nc.dram_tensor("out", [n_sb, SB], f32, kind="ExternalOutput")

    do_gather = mode in ("all", "gather")
    do_compute = mode in ("all", "compute")

    with tile.TileContext(nc) as tc, ExitStack() as ctx:
        const = ctx.enter_context(tc.tile_pool(name="const", bufs=1))
        gpool = ctx.enter_context(tc.tile_pool(name="gath", bufs=3))
        tpool = ctx.enter_context(tc.tile_pool(name="xT", bufs=3))
        hpool = ctx.enter_context(tc.tile_pool(name="h", bufs=3))
        spool = ctx.enter_context(tc.tile_pool(name="s", bufs=4))
        psT = ctx.enter_context(tc.tile_pool(name="psT", bufs=2, space="PSUM"))
        psH = ctx.enter_context(tc.tile_pool(name="psH", bufs=2, space="PSUM"))
        psS = ctx.enter_context(tc.tile_pool(name="psS", bufs=2, space="PSUM"))

        ident = const.tile([P, P], f32)
        make_identity(nc, ident[:])
        w1a_t = const.tile([D, D], f32)
        nc.sync.dma_start(w1a_t[:], w1a_d[:, :])
        w1b_t = const.tile([D, D], f32)
        nc.sync.dma_start(w1b_t[:], w1b_d[:, :])
        b1_t = const.tile([D, 1], f32)
        nc.sync.dma_start(b1_t[:], b1_d[:, :])
        w2_t = const.tile([D, 1], f32)
        nc.sync.dma_start(w2_t[:], w2_d[:, :])
        b2_t = const.tile([1, 1], f32)
        nc.sync.dma_start(b2_t[:], b2_d[:, :])
        src_t = const.tile([P, S // 16], i16)
        nc.sync.dma_start(src_t[:], src_d[:, :])
        tar_t = const.tile([P, S // 16], i16)
        nc.sync.dma_start(tar_t[:], tar_d[:, :])

        x_win = [x_d[w * WIN : w * WIN + WLEN[w], :] for w in range(N_WIN)]

        def body():
            sb_global = 0
            slot_off = 0
            qrr = [0]
            for ws in range(N_WIN):
                for wt in range(N_WIN):
                    cap = caps[ws * N_WIN + wt]
                    for g in gather_split(cap):
                        C = g // P
                        xs_g = gpool.tile([P, C, D], f32, tag="xs")
                        xt_g = gpool.tile([P, C, D], f32, tag="xt")
                        if do_gather:
                            nc.gpsimd.dma_gather(
                                xs_g[:, :, :],
                                x_win[ws],
                                src_t[:, slot_off // 16 : (slot_off + g) // 16],
                                g,
                                g,
                                D,
                                queue_num=qrr[0] % 4,
                                single_packet=False,
                            )
                            qrr[0] += 1
                            nc.gpsimd.dma_gather(
                                xt_g[:, :, :],
                                x_win[wt],
                                tar_t[:, slot_off // 16 : (slot_off + g) // 16],
                                g,
                                g,
                                D,
                                queue_num=qrr[0] % 4,
                                single_packet=False,
                            )
                            qrr[0] += 1
                        if do_compute:
                            for s in range(g // SB):
                                xsT_ps = psT.tile([P, SB], f32, tag="xsT")
                                xtT_ps = psT.tile([P, SB], f32, tag="xtT")
                                for c in range(4):
                                    nc.tensor.transpose(
                                        out=xsT_ps[:, c * P : (c + 1) * P],
                                        in_=xs_g[:, 4 * s + c, :],
                                        identity=ident[:],
                                    )
                                    nc.tensor.transpose(
                                        out=xtT_ps[:, c * P : (c + 1) * P],
                                        in_=xt_g[:, 4 * s + c, :],
                                        identity=ident[:],
                                    )
                                xsT = tpool.tile([P, SB], f32, tag="xsTs")
                                xtT = tpool.tile([P, SB], f32, tag="xtTs")
                                nc.vector.tensor_copy(xsT[:], xsT_ps[:])
                                nc.vector.tensor_copy(xtT[:], xtT_ps[:])

                                h_ps = psH.tile([P, SB], f32, tag="h")
                                nc.tensor.matmul(
                                    h_ps[:], lhsT=w1a_t[:], rhs=xsT[:],
                                    start=True, stop=False,
                                )
                                nc.tensor.matmul(
                                    h_ps[:], lhsT=w1b_t[:], rhs=xtT[:],
                                    start=False, stop=True,
                                )
                                hrelu = hpool.tile([P, SB], f32, tag="hrelu")
                                nc.scalar.activation(
                                    out=hrelu[:], in_=h_ps[:], func=FT.Relu,
                                    bias=b1_t[:, 0:1],
                                )
                                s_ps = psS.tile([1, SB], f32, tag="s")
                                nc.tensor.matmul(
                                    s_ps[:], lhsT=w2_t[:], rhs=hrelu[:],
                                    start=True, stop=True,
                                )
                                s_sb = spool.tile([1, SB], f32, tag="sout")
                                nc.scalar.activation(
                                    out=s_sb[:], in_=s_ps[:], func=FT.Identity,
                                    bias=b2_t[0:1, 0:1],
                                )
                                nc.sync.dma_start(
                                    out_d[sb_global : sb_global + 1, :], s_sb[0:1, :]
                                )
                                sb_global += 1
                        slot_off += g

        loop_cm = tc.For_i(0, hw_loop, 1) if hw_loop else nullcontext()
        with loop_cm:
            for _ in range(repeat):
                body()

    # Tile assigns Pool DMAs to DMASW sem lanes round-robin in *scheduled*
    # order; a DMA semaphore may only be used by one SWDGE queue. Rewrite each
    # gather's queue_num to follow its assigned lane so sem<->queue stays
    # consistent (and the 4 ucode queues are load balanced).
    from concourse.tile_scheduler import PROC_NAME_TO_IDX

    lane_of = {PROC_NAME_TO_IDX[f"DMASW{k}"]: k for k in range(8)}
    for f in nc.m.functions:
        for blk in f.blocks:
            for inst in blk.instructions:
                if isinstance(inst, mybir.InstDMAGatherAnt):
                    inst.queue_num = lane_of[inst.bass_scheduled_proc] % 4

    nc.compile()
    return nc


def prep_core(src, tar, caps):
    """Bucket one core's edges; returns wrapped int16 idx tables and the
    slot index of each edge (or None on capacity overflow)."""
    n_edges = len(src)
    S = int(sum(caps))
    ws = src >> 15
    wt = tar >> 15
    b = ws * N_WIN + wt
    sizes = np.bincount(b, minlength=16)
    if np.any(sizes > np.asarray(caps)):
        return None
    order = np.argsort(b, kind="stable")
    base = np.concatenate([[0], np.cumsum(caps)]).astype(np.int64)
    cum = np.concatenate([[0], np.cumsum(sizes)]).astype(np.int64)
    vsrc = np.zeros(S, np.int16)
    vtar = np.zeros(S, np.int16)
    slot_of_edge = np.empty(n_edges, np.int64)
    for bb in range(16):
        e = order[cum[bb] : cum[bb + 1]]
        slots = base[bb] + np.arange(len(e))
        slot_of_edge[e] = slots
        vsrc[slots] = (src[e] & 32767).astype(np.int16)
        vtar[slots] = (tar[e] & 32767).astype(np.int16)

    def wrap(v):
        t = v.reshape(S // 16, 16).T  # [16, S/16]
        return np.ascontiguousarray(np.tile(t, (P // 16, 1)))

    return wrap(vsrc), wrap(vtar), slot_of_edge


_CACHE = {}


def _get_nc(caps):
    key = ("nc", caps)
    if key not in _CACHE:
        _CACHE[key] = build_nc(caps)
    return _CACHE[key]


def kernel(**inputs):
    x = np.ascontiguousarray(np.asarray(inputs["x"], dtype=np.float32))
    pos = np.asarray(inputs["pos_edge_index"])
    neg = np.asarray(inputs["neg_edge_index"])
    W1 = np.asarray(inputs["W1"], dtype=np.float32)
    b1 = np.asarray(inputs["b1"], dtype=np.float32)
    W2 = np.asarray(inputs["W2"], dtype=np.float32)
    b2 = np.asarray(inputs["b2"], dtype=np.float32)

    edge = np.concatenate([pos, neg], axis=1).astype(np.int64)  # [2, E_TOTAL]
    src, tar = edge[0], edge[1]

    caps = default_caps()
    preps = []
    for c in range(N_CORES):
        lo, hi = c * E_PER_CORE, (c + 1) * E_PER_CORE
        pr = prep_core(src[lo:hi], tar[lo:hi], caps)
        if pr is None:
            # capacity overflow (shouldn't happen for uniform random inputs):
            # rebuild with actual sizes + slack
            sizes = np.zeros(16, np.int64)
            for cc in range(N_CORES):
                l2, h2 = cc * E_PER_CORE, (cc + 1) * E_PER_CORE
                bb = (src[l2:h2] >> 15) * N_WIN + (tar[l2:h2] >> 15)
                sizes = np.maximum(sizes, np.bincount(bb, minlength=16))
            caps = tuple(
                int(np.ceil((s + 256) / SB)) * SB for s in sizes
            )
            preps = []
            for cc in range(N_CORES):
                l2, h2 = cc * E_PER_CORE, (cc + 1) * E_PER_CORE
                preps.append(prep_core(src[l2:h2], tar[l2:h2], caps))
            break
        preps.append(pr)

    nc = _get_nc(caps)

    w1a = np.ascontiguousarray(W1[:D, :])
    w1b = np.ascontiguousarray(W1[D:, :])
    b1c = np.ascontiguousarray(b1.reshape(D, 1))
    w2c = np.ascontiguousarray(W2.reshape(D, 1))
    b2c = np.ascontiguousarray(b2.reshape(1, 1))

    in_maps = []
    for c in range(N_CORES):
        vsrc, vtar, _ = preps[c]
        in_maps.append(
            {
                "x": x,
                "src": vsrc,
                "tar": vtar,
                "w1a": w1a,
                "w1b": w1b,
                "b1": b1c,
                "w2": w2c,
                "b2": b2c,
            }
        )

    from concourse.bass_utils import run_bass_kernel_spmd

    _CACHE["in_maps"] = in_maps
    _CACHE["caps"] = caps
    res = run_bass_kernel_spmd(nc, in_maps, list(range(N_CORES))).results
    out = np.empty((E_TOTAL,), np.float32)
    for c in range(N_CORES):
        flat = res[c]["out"].reshape(-1)
        lo = c * E_PER_CORE
        out[lo : lo + E_PER_CORE] = flat[preps[c][2]]
    return out.reshape(E_TOTAL, 1).astype(np.float32)


if __name__ == "__main__":
    rng = np.random.default_rng(0)
    ins = {
        "x": rng.standard_normal((N_NODES, D), dtype=np.float32),
        "pos_edge_index": rng.integers(0, N_NODES, (2, E_TOTAL // 2)),
        "neg_edge_index": rng.integers(0, N_NODES, (2, E_TOTAL // 2)),
        "W1": rng.standard_normal((2 * D, D), dtype=np.float32) * 0.06,
        "b1": rng.standard_normal(D, dtype=np.float32) * 0.06,
        "W2": rng.standard_normal((D, 1), dtype=np.float32) * 0.09,
        "b2": rng.standard_normal(1, dtype=np.float32) * 0.09,
    }
    out = kernel(**ins)
    s = np.concatenate([ins["pos_edge_index"][0], ins["neg_edge_index"][0]])
    t = np.concatenate([ins["pos_edge_index"][1], ins["neg_edge_index"][1]])
    h = np.maximum(ins["x"][s] @ ins["W1"][:D] + ins["x"][t] @ ins["W1"][D:] + ins["b1"], 0.0)
    exp = h @ ins["W2"] + ins["b2"]
    err = np.abs(out - exp).max() / max(np.abs(exp).max(), 1e-9)
    print("max rel err:", err)


# revision 23
# speedup vs baseline: 19.7993x; 1.0278x over previous
"""Trainium2 Bass kernel for nn_Decoder (GNN edge decoder / link predictor).

Math (per edge e with endpoints src[e], tar[e]):
    h   = relu(x[src] @ W1[:D] + x[tar] @ W1[D:] + b1)        # [E, D]
    out = h @ W2 + b2                                          # [E, 1]

Strategy (8 NeuronCores, SPMD):
  - Shard the 524288 edges across 8 cores (65536 edges each); replicate x
    and the MLP weights on every core. No collectives.
  - Random row gather is the bottleneck. SWDGE has ~1us fixed cost per DMA
    instruction, so per-128-row indirect DMAs are too slow. Instead use the
    dma_gather ucode (InstDMAGatherAnt) with 2048 rows per instruction.
    dma_gather indices are int16 (max 32767), so the host buckets each
    core's edges by the (src_window, tar_window) pair, where a window is a
    32768-row slice of x. Each bucket has a static capacity (so the program
    is input-independent); pad slots gather row 0 and are discarded.
  - Per 512-edge superblock: PE-transpose 128x128 blocks -> [feature, edge]
    layout, hT = W1a.T@xsT + W1b.T@xtT (weights stationary, PSUM accum),
    relu(hT + b1) on ACT (per-partition bias), scores = W2.T @ relu_hT on
    PE, + b2 on ACT, DMA out.
  - Host maps device slots back to original edge order at the end.
"""

import sys
from contextlib import ExitStack, nullcontext

import numpy as np

if "/opt/trn_rl_repo" not in sys.path:
    sys.path.insert(0, "/opt/trn_rl_repo")

N_NODES = 100000
D = 128
E_TOTAL = 524288
N_CORES = 8
E_PER_CORE = E_TOTAL // N_CORES  # 65536
SB = 512  # edges per superblock
P = 128
WIN = 32768  # index window (int16 range)
N_WIN = 4  # ceil(100000 / 32768)
GIDX = 2048  # rows per dma_gather instruction (single_packet=False)
WLEN = [WIN, WIN, WIN, N_NODES - 3 * WIN]  # rows per window


def default_caps(n_edges=E_PER_CORE):
    """Static per-bucket slot capacities (multiples of SB), sized at
    mean + ~6 sigma for uniform random endpoints."""
    pw = np.array([WLEN[0], WLEN[1], WLEN[2], WLEN[3]], np.float64) / N_NODES
    caps = []
    for ws in range(N_WIN):
        for wt in range(N_WIN):
            pb = pw[ws] * pw[wt]
            mean = n_edges * pb
            std = np.sqrt(n_edges * pb * (1 - pb))
            need = mean + 6.0 * std + 8
            caps.append(max(SB, int(np.ceil(need / SB)) * SB))
    return tuple(caps)


def gather_split(cap):
    """Split a bucket capacity into dma_gather instruction sizes."""
    out = []
    while cap > 0:
        g = min(GIDX, cap)
        out.append(g)
        cap -= g
    return out


def build_nc(caps, repeat=1, x_external=True, hw_loop=0, mode="all", gbufs=4):
    import concourse.bacc as bacc
    import concourse.bass as bass
    import concourse.mybir as mybir
    import concourse.tile as tile
    from concourse.masks import make_identity

    f32 = mybir.dt.float32
    i16 = mybir.dt.int16
    FT = mybir.ActivationFunctionType

    S = int(sum(caps))
    n_sb = S // SB

    nc = bacc.Bacc("TRN2", target_bir_lowering=False, debug=False, num_swdge_queues=4)
    if x_external:
        x_d = nc.dram_tensor("x", [N_NODES, D], f32, kind="ExternalInput")
    else:
        x_d = nc.dram_tensor("x", [N_NODES, D], f32)
    # wrapped int16 index tables: [p, j] = local_idx of slot (j*16 + p%16)
    src_d = nc.dram_tensor("src", [P, S // 16], i16, kind="ExternalInput")
    tar_d = nc.dram_tensor("tar", [P, S // 16], i16, kind="ExternalInput")
    w1a_d = nc.dram_tensor("w1a", [D, D], f32, kind="ExternalInput")
    w1b_d = nc.dram_tensor("w1b", [D, D], f32, kind="ExternalInput")
    b1_d = nc.dram_tensor("b1", [D, 1], f32, kind="ExternalInput")
    w2_d = nc.dram_tensor("w2", [D, 1], f32, kind="ExternalInput")
    b2_d = nc.dram_tensor("b2", [1, 1], f32, kind="ExternalInput")
    out_d = nc.dram_tensor("out", [n_sb, SB], f32, kind="ExternalOutput")

    do_gather = mode in ("all", "gather")
    do_compute = mode in ("all", "compute")

    with tile.TileContext(nc) as tc, ExitStack() as ctx:
        const = ctx.enter_context(tc.tile_pool(name="const", bufs=1))
        gpool = ctx.enter_context(tc.tile_pool(name="gath", bufs=gbufs))
        tpool = ctx.enter_context(tc.tile_pool(name="xT", bufs=3))
        hpool = ctx.enter_context(tc.tile_pool(name="h", bufs=3))
        spool = ctx.enter_context(tc.tile_pool(name="s", bufs=4))
        psT = ctx.enter_context(tc.tile_pool(name="psT", bufs=2, space="PSUM"))
        psH = ctx.enter_context(tc.tile_pool(name="psH", bufs=2, space="PSUM"))
        psS = ctx.enter_context(tc.tile_pool(name="psS", bufs=2, space="PSUM"))

        ident = const.tile([P, P], f32)
        make_identity(nc, ident[:])
        w1a_t = const.tile([D, D], f32)
        nc.sync.dma_start(w1a_t[:], w1a_d[:, :])
        w1b_t = const.tile([D, D], f32)
        nc.sync.dma_start(w1b_t[:], w1b_d[:, :])
        b1_t = const.tile([D, 1], f32)
        nc.sync.dma_start(b1_t[:], b1_d[:, :])
        w2_t = const.tile([D, 1], f32)
        nc.sync.dma_start(w2_t[:], w2_d[:, :])
        b2_t = const.tile([1, 1], f32)
        nc.sync.dma_start(b2_t[:], b2_d[:, :])
        src_t = const.tile([P, S // 16], i16)
        nc.sync.dma_start(src_t[:], src_d[:, :])
        tar_t = const.tile([P, S // 16], i16)
        nc.sync.dma_start(tar_t[:], tar_d[:, :])

        x_win = [x_d[w * WIN : w * WIN + WLEN[w], :] for w in range(N_WIN)]

        def body():
            sb_global = 0
            slot_off = 0
            qrr = [0]
            for ws in range(N_WIN):
                for wt in range(N_WIN):
                    cap = caps[ws * N_WIN + wt]
                    for g in gather_split(cap):
                        C = g // P
                        xs_g = gpool.tile([P, C, D], f32, tag="xs")
                        xt_g = gpool.tile([P, C, D], f32, tag="xt")
                        if do_gather:
                            nc.gpsimd.dma_gather(
                                xs_g[:, :, :],
                                x_win[ws],
                                src_t[:, slot_off // 16 : (slot_off + g) // 16],
                                g,
                                g,
                                D,
                                queue_num=qrr[0] % 4,
                                single_packet=False,
                            )
                            qrr[0] += 1
                            nc.gpsimd.dma_gather(
                                xt_g[:, :, :],
                                x_win[wt],
                                tar_t[:, slot_off // 16 : (slot_off + g) // 16],
                                g,
                                g,
                                D,
                                queue_num=qrr[0] % 4,
                                single_packet=False,
                            )
                            qrr[0] += 1
                        if do_compute:
                            for s in range(g // SB):
                                xsT_ps = psT.tile([P, SB], f32, tag="xsT")
                                xtT_ps = psT.tile([P, SB], f32, tag="xtT")
                                for c in range(4):
                                    nc.tensor.transpose(
                                        out=xsT_ps[:, c * P : (c + 1) * P],
                                        in_=xs_g[:, 4 * s + c, :],
                                        identity=ident[:],
                                    )
                                    nc.tensor.transpose(
                                        out=xtT_ps[:, c * P : (c + 1) * P],
                                        in_=xt_g[:, 4 * s + c, :],
                                        identity=ident[:],
                                    )
                                xsT = tpool.tile([P, SB], f32, tag="xsTs")
                                xtT = tpool.tile([P, SB], f32, tag="xtTs")
                                nc.vector.tensor_copy(xsT[:], xsT_ps[:])
                                nc.vector.tensor_copy(xtT[:], xtT_ps[:])

                                h_ps = psH.tile([P, SB], f32, tag="h")
                                nc.tensor.matmul(
                                    h_ps[:], lhsT=w1a_t[:], rhs=xsT[:],
                                    start=True, stop=False,
                                )
                                nc.tensor.matmul(
                                    h_ps[:], lhsT=w1b_t[:], rhs=xtT[:],
                                    start=False, stop=True,
                                )
                                hrelu = hpool.tile([P, SB], f32, tag="hrelu")
                                nc.scalar.activation(
                                    out=hrelu[:], in_=h_ps[:], func=FT.Relu,
                                    bias=b1_t[:, 0:1],
                                )
                                s_ps = psS.tile([1, SB], f32, tag="s")
                                nc.tensor.matmul(
                                    s_ps[:], lhsT=w2_t[:], rhs=hrelu[:],
                                    start=True, stop=True,
                                )
                                s_sb = spool.tile([1, SB], f32, tag="sout")
                                nc.scalar.activation(
                                    out=s_sb[:], in_=s_ps[:], func=FT.Identity,
                                    bias=b2_t[0:1, 0:1],
                                )
                                nc.sync.dma_start(
                                    out_d[sb_global : sb_global + 1, :], s_sb[0:1, :]
                                )
                                sb_global += 1
                        slot_off += g

        loop_cm = tc.For_i(0, hw_loop, 1) if hw_loop else nullcontext()
        with loop_cm:
            for _ in range(repeat):
                body()

    # Tile assigns Pool DMAs to DMASW sem lanes round-robin in *scheduled*
    # order; a DMA semaphore may only be used by one SWDGE queue. Rewrite each
    # gather's queue_num to follow its assigned lane so sem<->queue stays
    # consistent (and the 4 ucode queues are load balanced).
    from concourse.tile_scheduler import PROC_NAME_TO_IDX

    lane_of = {PROC_NAME_TO_IDX[f"DMASW{k}"]: k for k in range(8)}
    for f in nc.m.functions:
        for blk in f.blocks:
            for inst in blk.instructions:
                if isinstance(inst, mybir.InstDMAGatherAnt):
                    inst.queue_num = lane_of[inst.bass_scheduled_proc] % 4

    nc.compile()
    return nc


def prep_core(src, tar, caps):
    """Bucket one core's edges; returns wrapped int16 idx tables and the
    slot index of each edge (or None on capacity overflow)."""
    n_edges = len(src)
    S = int(sum(caps))
    ws = src >> 15
    wt = tar >> 15
    b = ws * N_WIN + wt
    sizes = np.bincount(b, minlength=16)
    if np.any(sizes > np.asarray(caps)):
        return None
    order = np.argsort(b, kind="stable")
    base = np.concatenate([[0], np.cumsum(caps)]).astype(np.int64)
    cum = np.concatenate([[0], np.cumsum(sizes)]).astype(np.int64)
    vsrc = np.zeros(S, np.int16)
    vtar = np.zeros(S, np.int16)
    slot_of_edge = np.empty(n_edges, np.int64)
    for bb in range(16):
        e = order[cum[bb] : cum[bb + 1]]
        slots = base[bb] + np.arange(len(e))
        slot_of_edge[e] = slots
        vsrc[slots] = (src[e] & 32767).astype(np.int16)
        vtar[slots] = (tar[e] & 32767).astype(np.int16)

    def wrap(v):
        t = v.reshape(S // 16, 16).T  # [16, S/16]
        return np.ascontiguousarray(np.tile(t, (P // 16, 1)))

    return wrap(vsrc), wrap(vtar), slot_of_edge


_CACHE = {}


def _get_nc(caps):
    key = ("nc", caps)
    if key not in _CACHE:
        _CACHE[key] = build_nc(caps)
    return _CACHE[key]


def kernel(**inputs):
    x = np.ascontiguousarray(np.asarray(inputs["x"], dtype=np.float32))
    pos = np.asarray(inputs["pos_edge_index"])
    neg = np.asarray(inputs["neg_edge_index"])
    W1 = np.asarray(inputs["W1"], dtype=np.float32)
    b1 = np.asarray(inputs["b1"], dtype=np.float32)
    W2 = np.asarray(inputs["W2"], dtype=np.float32)
    b2 = np.asarray(inputs["b2"], dtype=np.float32)

    edge = np.concatenate([pos, neg], axis=1).astype(np.int64)  # [2, E_TOTAL]
    src, tar = edge[0], edge[1]

    caps = default_caps()
    preps = []
    for c in range(N_CORES):
        lo, hi = c * E_PER_CORE, (c + 1) * E_PER_CORE
        pr = prep_core(src[lo:hi], tar[lo:hi], caps)
        if pr is None:
            # capacity overflow (shouldn't happen for uniform random inputs):
            # rebuild with actual sizes + slack
            sizes = np.zeros(16, np.int64)
            for cc in range(N_CORES):
                l2, h2 = cc * E_PER_CORE, (cc + 1) * E_PER_CORE
                bb = (src[l2:h2] >> 15) * N_WIN + (tar[l2:h2] >> 15)
                sizes = np.maximum(sizes, np.bincount(bb, minlength=16))
            caps = tuple(
                int(np.ceil((s + 256) / SB)) * SB for s in sizes
            )
            preps = []
            for cc in range(N_CORES):
                l2, h2 = cc * E_PER_CORE, (cc + 1) * E_PER_CORE
                preps.append(prep_core(src[l2:h2], tar[l2:h2], caps))
            break
        preps.append(pr)

    nc = _get_nc(caps)

    w1a = np.ascontiguousarray(W1[:D, :])
    w1b = np.ascontiguousarray(W1[D:, :])
    b1c = np.ascontiguousarray(b1.reshape(D, 1))
    w2c = np.ascontiguousarray(W2.reshape(D, 1))
    b2c = np.ascontiguousarray(b2.reshape(1, 1))

    in_maps = []
    for c in range(N_CORES):
        vsrc, vtar, _ = preps[c]
        in_maps.append(
            {
                "x": x,
                "src": vsrc,
                "tar": vtar,
                "w1a": w1a,
                "w1b": w1b,
                "b1": b1c,
                "w2": w2c,
                "b2": b2c,
            }
        )

    from concourse.bass_utils import run_bass_kernel_spmd

    _CACHE["in_maps"] = in_maps
    _CACHE["caps"] = caps
    res = run_bass_kernel_spmd(nc, in_maps, list(range(N_CORES))).results
    out = np.empty((E_TOTAL,), np.float32)
    for c in range(N_CORES):
        flat = res[c]["out"].reshape(-1)
        lo = c * E_PER_CORE
        out[lo : lo + E_PER_CORE] = flat[preps[c][2]]
    return out.reshape(E_TOTAL, 1).astype(np.float32)


if __name__ == "__main__":
    rng = np.random.default_rng(0)
    ins = {
        "x": rng.standard_normal((N_NODES, D), dtype=np.float32),
        "pos_edge_index": rng.integers(0, N_NODES, (2, E_TOTAL // 2)),
        "neg_edge_index": rng.integers(0, N_NODES, (2, E_TOTAL // 2)),
        "W1": rng.standard_normal((2 * D, D), dtype=np.float32) * 0.06,
        "b1": rng.standard_normal(D, dtype=np.float32) * 0.06,
        "W2": rng.standard_normal((D, 1), dtype=np.float32) * 0.09,
        "b2": rng.standard_normal(1, dtype=np.float32) * 0.09,
    }
    out = kernel(**ins)
    s = np.concatenate([ins["pos_edge_index"][0], ins["neg_edge_index"][0]])
    t = np.concatenate([ins["pos_edge_index"][1], ins["neg_edge_index"][1]])
    h = np.maximum(ins["x"][s] @ ins["W1"][:D] + ins["x"][t] @ ins["W1"][D:] + ins["b1"], 0.0)
    exp = h @ ins["W2"] + ins["b2"]
    err = np.abs(out - exp).max() / max(np.abs(exp).max(), 1e-9)
    print("max rel err:", err)
